# revision 1
# baseline (speedup 1.0000x reference)
"""CentroidAlignmentLoss on 8 TRN2 NeuronCores (Bass/Tile, SPMD).

Math: with per-class counts n_c, sums s_c = sum_{i in c} e_i and
sumsq_c = sum_{i in c} ||e_i||^2, the reference's per-class mean squared
distance to the centroid mu_c = s_c / max(n_c, 1) satisfies (exactly, for
n_c > 0):
    sum_{i in c} ||e_i - mu_c||^2 = sumsq_c - ||s_c||^2 / n_c
so  loss = (1/U) * sum_{c: n_c>0} [ sumsq_c / n_c - ||s_c||^2 / n_c^2 ],
with U the number of non-empty classes. Empty classes contribute 0 in both
forms. This needs only ONE pass over the embeddings.

Device strategy (data-parallel over rows, all 100 classes per core):
  - one-hot H for each 128-row group via DVE is_equal(iota, label_col)
  - TensorE accumulates H^T @ E -> sums[C,D] and H^T @ [E^2 | 1] ->
    (per-dim sumsq | counts)[C,D+1] in PSUM across all groups
  - AllReduce the [C, 2D+1] partials over the 8 cores
  - tiny on-device reduction to the scalar loss
"""

import numpy as np

import concourse.bacc as bacc
import concourse.mybir as mybir
import concourse.tile as tile
from concourse.alu_op_type import AluOpType
from concourse.bass_utils import run_bass_kernel_spmd

N = 262144
D = 256
C = 100
N_CORES = 8
SHARD = N // N_CORES      # 32768 rows per core
P = 128                   # rows per matmul group (= SBUF partitions)
G = 8                     # groups per super-tile (1 MB DMAs)
ROWS_ST = P * G           # 1024 rows per super-tile
N_ST = SHARD // ROWS_ST   # 32 super-tiles per core
LCOLS = SHARD // P        # 256 label columns per core

_cache = {}


def _build():
    f32 = mybir.dt.float32
    nc = bacc.Bacc("TRN2", target_bir_lowering=False, debug=False,
                   num_devices=N_CORES)
    emb = nc.dram_tensor("emb", [SHARD, D], f32, kind="ExternalInput")
    labt = nc.dram_tensor("labt", [P, LCOLS], f32, kind="ExternalInput")
    iota = nc.dram_tensor("iota", [P, C], f32, kind="ExternalInput")
    loss = nc.dram_tensor("loss", [1, 1], f32, kind="ExternalOutput")

    # emb row (t*1024 + p*8 + g) -> super-tile t, partition p, group g.
    # Per partition the DMA source is 8 KB contiguous.
    emb_v = emb[:].rearrange("(t p g) d -> t p (g d)", p=P, g=G)

    with tile.TileContext(nc) as tc:
        with tc.tile_pool(name="const", bufs=1) as const, \
             tc.tile_pool(name="esup", bufs=3) as esup_pool, \
             tc.tile_pool(name="esq", bufs=3) as esq_pool, \
             tc.tile_pool(name="hpool", bufs=8) as hpool, \
             tc.tile_pool(name="psum", bufs=1, space="PSUM") as psum_pool, \
             tc.tile_pool(name="dram", bufs=1, space="DRAM") as dram_pool:

            iota_sb = const.tile([P, C], f32)
            nc.sync.dma_start(iota_sb[:], iota[:])
            labt_sb = const.tile([P, LCOLS], f32)
            nc.sync.dma_start(labt_sb[:], labt[:])

            psum_sums = psum_pool.tile([C, D], f32)       # H^T @ E
            psum_sq = psum_pool.tile([C, D + 1], f32)     # H^T @ [E^2 | 1]

            for t in range(N_ST):
                e_sup = esup_pool.tile([P, G * D], f32)
                nc.sync.dma_start(e_sup[:], emb_v[t])
                esq = esq_pool.tile([P, G, D + 1], f32)
                nc.scalar.activation(
                    esq[:, :, 0:D],
                    e_sup[:].rearrange("p (g d) -> p g d", g=G),
                    mybir.ActivationFunctionType.Square,
                )
                nc.any.memset(esq[:, :, D], 1.0)
                for g in range(G):
                    h = hpool.tile([P, C], f32)
                    nc.vector.tensor_scalar(
                        h[:], iota_sb[:],
                        labt_sb[:, t * G + g: t * G + g + 1],
                        None, AluOpType.is_equal,
                    )
                    first = t == 0 and g == 0
                    last = t == N_ST - 1 and g == G - 1
                    nc.tensor.matmul(psum_sums[:], h[:],
                                     e_sup[:, g * D:(g + 1) * D],
                                     start=first, stop=last)
                    nc.tensor.matmul(psum_sq[:], h[:], esq[:, g, :],
                                     start=first, stop=last)

            # Pack partial stats [C, 2D+1] = [sums | per-dim sumsq | counts]
            stats = const.tile([C, 2 * D + 1], f32)
            nc.vector.tensor_copy(stats[:, 0:D], psum_sums[:])
            nc.vector.tensor_copy(stats[:, D:2 * D + 1], psum_sq[:])

            partial = dram_pool.tile([C, 2 * D + 1], f32)
            allred = dram_pool.tile([C, 2 * D + 1], f32)
            nc.sync.dma_start(partial[:], stats[:])
            nc.gpsimd.collective_compute(
                "AllReduce", AluOpType.add,
                replica_groups=[list(range(N_CORES))],
                ins=[partial.opt()], outs=[allred.opt()],
            )
            red_sb = const.tile([C, 2 * D + 1], f32)
            nc.sync.dma_start(red_sb[:], allred[:])

            sums = red_sb[:, 0:D]
            sumsq_cols = red_sb[:, D:2 * D]
            counts = red_sb[:, 2 * D:2 * D + 1]

            sumsq = const.tile([C, 1], f32)
            nc.vector.tensor_reduce(sumsq[:], sumsq_cols,
                                    axis=mybir.AxisListType.X,
                                    op=AluOpType.add)
            sq_scr = const.tile([C, D], f32)
            s2 = const.tile([C, 1], f32)   # ||s_c||^2
            nc.scalar.activation(sq_scr[:], sums,
                                 mybir.ActivationFunctionType.Square,
                                 accum_out=s2[:])
            safe = const.tile([C, 1], f32)
            nc.vector.tensor_scalar_max(safe[:], counts, 1.0)
            inv = const.tile([C, 1], f32)
            nc.vector.reciprocal(inv[:], safe[:])

            # per-class loss = (sumsq - s2*inv) * inv ; 0 for empty classes
            pf = const.tile([C, 2], f32)
            t1 = const.tile([C, 1], f32)
            nc.vector.tensor_tensor(t1[:], s2[:], inv[:], AluOpType.mult)
            t2 = const.tile([C, 1], f32)
            nc.vector.tensor_tensor(t2[:], sumsq[:], t1[:], AluOpType.subtract)
            nc.vector.tensor_tensor(pf[:, 0:1], t2[:], inv[:], AluOpType.mult)
            nc.vector.tensor_scalar(pf[:, 1:2], counts, 0.0, None,
                                    AluOpType.is_gt)

            # partition-sum via ones^T @ [per | flag] -> [1, 2]
            ones_col = const.tile([P, 1], f32)
            nc.any.memset(ones_col[:], 1.0)
            fin_ps = psum_pool.tile([1, 2], f32)
            nc.tensor.matmul(fin_ps[:], ones_col[:C, :], pf[:],
                             start=True, stop=True)
            fin = const.tile([1, 2], f32)
            nc.vector.tensor_copy(fin[:], fin_ps[:])
            r = const.tile([1, 1], f32)
            nc.vector.reciprocal(r[:], fin[:, 1:2])
            out_sb = const.tile([1, 1], f32)
            nc.vector.tensor_tensor(out_sb[:], fin[:, 0:1], r[:],
                                    AluOpType.mult)
            nc.sync.dma_start(loss[:], out_sb[:])

    nc.compile()
    return nc


def _in_maps(embeddings: np.ndarray, labels: np.ndarray):
    emb = np.ascontiguousarray(np.asarray(embeddings), dtype=np.float32)
    lab = np.asarray(labels).astype(np.float32)
    iota = np.ascontiguousarray(
        np.broadcast_to(np.arange(C, dtype=np.float32), (P, C)))
    maps = []
    for i in range(N_CORES):
        sl = slice(i * SHARD, (i + 1) * SHARD)
        labt = np.ascontiguousarray(
            lab[sl].reshape(N_ST, P, G).transpose(1, 0, 2).reshape(P, LCOLS))
        maps.append({"emb": emb[sl], "labt": labt, "iota": iota})
    return maps


def _run(embeddings: np.ndarray, labels: np.ndarray, trace: bool = False):
    if "nc" not in _cache:
        _cache["nc"] = _build()
    nc = _cache["nc"]
    res = run_bass_kernel_spmd(nc, _in_maps(embeddings, labels),
                               list(range(N_CORES)), trace=trace)
    out = np.float32(res.results[0]["loss"][0, 0])
    return out.reshape(()), res


def kernel(embeddings: np.ndarray, labels: np.ndarray) -> np.ndarray:
    out, _ = _run(embeddings, labels)
    return out


# revision 4
# speedup vs baseline: 1.5137x; 1.5137x over previous
"""CentroidAlignmentLoss on 8 TRN2 NeuronCores (Bass/Tile, SPMD).

Math: with per-class counts n_c, sums s_c = sum_{i in c} e_i and
sumsq_c = sum_{i in c} ||e_i||^2, the reference's per-class mean squared
distance to the centroid mu_c = s_c / max(n_c, 1) satisfies (exactly, for
n_c > 0):
    sum_{i in c} ||e_i - mu_c||^2 = sumsq_c - ||s_c||^2 / n_c
so  loss = (1/U) * sum_{c: n_c>0} [ sumsq_c / n_c - ||s_c||^2 / n_c^2 ],
with U the number of non-empty classes. Empty classes contribute 0 in both
forms. This needs only ONE pass over the embeddings.

Device strategy (data-parallel over rows, all 100 classes per core):
  - one-hot H for each 128-row group via DVE is_equal(iota, label_col)
  - TensorE accumulates H^T @ E -> sums[C,D] and H^T @ [E^2 | 1] ->
    (per-dim sumsq | counts)[C,D+1] in PSUM across all groups
  - AllReduce the [C, 2D+1] partials over the 8 cores
  - tiny on-device reduction to the scalar loss
"""

import numpy as np

import concourse.bacc as bacc
import concourse.mybir as mybir
import concourse.tile as tile
from concourse.alu_op_type import AluOpType
from concourse.bass_utils import run_bass_kernel_spmd

N = 262144
D = 256
C = 100
N_CORES = 8
SHARD = N // N_CORES      # 32768 rows per core
P = 128                   # rows per matmul group (= SBUF partitions)
G = 8                     # groups per super-tile (1 MB DMAs)
ROWS_ST = P * G           # 1024 rows per super-tile
N_ST = SHARD // ROWS_ST   # 32 super-tiles per core
LCOLS = SHARD // P        # 256 label columns per core

_cache = {}


def _build():
    f32 = mybir.dt.float32
    bf16 = mybir.dt.bfloat16
    nc = bacc.Bacc("TRN2", target_bir_lowering=False, debug=False,
                   num_devices=N_CORES)
    emb = nc.dram_tensor("emb", [SHARD, D], f32, kind="ExternalInput")
    labt = nc.dram_tensor("labt", [P, LCOLS], f32, kind="ExternalInput")
    iota = nc.dram_tensor("iota", [P, P], bf16, kind="ExternalInput")
    loss = nc.dram_tensor("loss", [1, 1], f32, kind="ExternalOutput")

    # emb row (t*1024 + p*8 + g) -> super-tile t, partition p, group g.
    # Per partition the DMA source is 8 KB contiguous.
    emb_v = emb[:].rearrange("(t p g) d -> t p (g d)", p=P, g=G)

    with tile.TileContext(nc) as tc:
        with tc.tile_pool(name="const", bufs=1) as const, \
             tc.tile_pool(name="esup", bufs=3) as esup_pool, \
             tc.tile_pool(name="ebf", bufs=3) as ebf_pool, \
             tc.tile_pool(name="esq", bufs=3) as esq_pool, \
             tc.tile_pool(name="hpool", bufs=8) as hpool, \
             tc.tile_pool(name="psum", bufs=1, space="PSUM") as psum_pool, \
             tc.tile_pool(name="dram", bufs=1, space="DRAM") as dram_pool:

            iota_sb = const.tile([P, P], bf16)
            nc.sync.dma_start(iota_sb[:], iota[:])
            labt_sb = const.tile([P, LCOLS], f32)
            nc.sync.dma_start(labt_sb[:], labt[:])

            # lhsT H is [P, P] (one-hot padded to 128 classes, cols C..127
            # never match) so bf16 FWL kicks in; PSUM rows C..127 stay zero.
            psum_sums = psum_pool.tile([P, D], f32)       # H^T @ E
            psum_sq = psum_pool.tile([P, D + 1], f32)     # H^T @ [E^2 | 1]

            for t in range(N_ST):
                e_sup = esup_pool.tile([P, G * D], f32)
                nc.sync.dma_start(e_sup[:], emb_v[t])
                e_bf = ebf_pool.tile([P, G * D], bf16)
                nc.vector.tensor_copy(e_bf[:], e_sup[:])
                esq = esq_pool.tile([P, G, D + 1], bf16)
                nc.scalar.activation(
                    esq[:, :, 0:D],
                    e_sup[:].rearrange("p (g d) -> p g d", g=G),
                    mybir.ActivationFunctionType.Square,
                )
                nc.any.memset(esq[:, :, D], 1.0)
                for g in range(G):
                    h = hpool.tile([P, P], bf16)
                    nc.vector.tensor_scalar(
                        h[:], iota_sb[:],
                        labt_sb[:, t * G + g: t * G + g + 1],
                        None, AluOpType.is_equal,
                    )
                    first = t == 0 and g == 0
                    last = t == N_ST - 1 and g == G - 1
                    nc.tensor.matmul(psum_sums[:], h[:],
                                     e_bf[:, g * D:(g + 1) * D],
                                     start=first, stop=last)
                    nc.tensor.matmul(psum_sq[:], h[:], esq[:, g, :],
                                     start=first, stop=last)

            # Pack partial stats [C, 2D+1] = [sums | per-dim sumsq | counts]
            stats = const.tile([C, 2 * D + 1], f32)
            nc.vector.tensor_copy(stats[:, 0:D], psum_sums[:C, :])
            nc.vector.tensor_copy(stats[:, D:2 * D + 1], psum_sq[:C, :])

            partial = dram_pool.tile([C, 2 * D + 1], f32)
            allred = dram_pool.tile([C, 2 * D + 1], f32)
            nc.sync.dma_start(partial[:], stats[:])
            nc.gpsimd.collective_compute(
                "AllReduce", AluOpType.add,
                replica_groups=[list(range(N_CORES))],
                ins=[partial.opt()], outs=[allred.opt()],
            )
            red_sb = const.tile([C, 2 * D + 1], f32)
            nc.sync.dma_start(red_sb[:], allred[:])

            sums = red_sb[:, 0:D]
            sumsq_cols = red_sb[:, D:2 * D]
            counts = red_sb[:, 2 * D:2 * D + 1]

            sumsq = const.tile([C, 1], f32)
            nc.vector.tensor_reduce(sumsq[:], sumsq_cols,
                                    axis=mybir.AxisListType.X,
                                    op=AluOpType.add)
            sq_scr = const.tile([C, D], f32)
            s2 = const.tile([C, 1], f32)   # ||s_c||^2
            nc.scalar.activation(sq_scr[:], sums,
                                 mybir.ActivationFunctionType.Square,
                                 accum_out=s2[:])
            safe = const.tile([C, 1], f32)
            nc.vector.tensor_scalar_max(safe[:], counts, 1.0)
            inv = const.tile([C, 1], f32)
            nc.vector.reciprocal(inv[:], safe[:])

            # per-class loss = (sumsq - s2*inv) * inv ; 0 for empty classes
            pf = const.tile([C, 2], f32)
            t1 = const.tile([C, 1], f32)
            nc.vector.tensor_tensor(t1[:], s2[:], inv[:], AluOpType.mult)
            t2 = const.tile([C, 1], f32)
            nc.vector.tensor_tensor(t2[:], sumsq[:], t1[:], AluOpType.subtract)
            nc.vector.tensor_tensor(pf[:, 0:1], t2[:], inv[:], AluOpType.mult)
            nc.vector.tensor_scalar(pf[:, 1:2], counts, 0.0, None,
                                    AluOpType.is_gt)

            # partition-sum via ones^T @ [per | flag] -> [1, 2]
            ones_col = const.tile([P, 1], f32)
            nc.any.memset(ones_col[:], 1.0)
            fin_ps = psum_pool.tile([1, 2], f32)
            nc.tensor.matmul(fin_ps[:], ones_col[:C, :], pf[:],
                             start=True, stop=True)
            fin = const.tile([1, 2], f32)
            nc.vector.tensor_copy(fin[:], fin_ps[:])
            r = const.tile([1, 1], f32)
            nc.vector.reciprocal(r[:], fin[:, 1:2])
            out_sb = const.tile([1, 1], f32)
            nc.vector.tensor_tensor(out_sb[:], fin[:, 0:1], r[:],
                                    AluOpType.mult)
            nc.sync.dma_start(loss[:], out_sb[:])

    nc.compile()
    return nc


def _in_maps(embeddings: np.ndarray, labels: np.ndarray):
    import ml_dtypes
    bf16 = ml_dtypes.bfloat16
    emb = np.ascontiguousarray(np.asarray(embeddings), dtype=np.float32)
    lab = np.asarray(labels).astype(np.float32)
    iota = np.ascontiguousarray(
        np.broadcast_to(np.arange(P, dtype=np.float32), (P, P))).astype(bf16)
    maps = []
    for i in range(N_CORES):
        sl = slice(i * SHARD, (i + 1) * SHARD)
        labt = np.ascontiguousarray(
            lab[sl].reshape(N_ST, P, G).transpose(1, 0, 2).reshape(P, LCOLS))
        maps.append({"emb": emb[sl], "labt": labt, "iota": iota})
    return maps


def _run(embeddings: np.ndarray, labels: np.ndarray, trace: bool = False):
    if "nc" not in _cache:
        _cache["nc"] = _build()
    nc = _cache["nc"]
    res = run_bass_kernel_spmd(nc, _in_maps(embeddings, labels),
                               list(range(N_CORES)), trace=trace)
    out = np.float32(res.results[0]["loss"][0, 0])
    return out.reshape(()), res


def kernel(embeddings: np.ndarray, labels: np.ndarray) -> np.ndarray:
    out, _ = _run(embeddings, labels)
    return out


# revision 5
# speedup vs baseline: 1.6137x; 1.0660x over previous
"""CentroidAlignmentLoss on 8 TRN2 NeuronCores (Bass/Tile, SPMD).

Math: with per-class counts n_c, sums s_c = sum_{i in c} e_i and
sumsq_c = sum_{i in c} ||e_i||^2, the reference's per-class mean squared
distance to the centroid mu_c = s_c / max(n_c, 1) satisfies (exactly, for
n_c > 0):
    sum_{i in c} ||e_i - mu_c||^2 = sumsq_c - ||s_c||^2 / n_c
so  loss = (1/U) * sum_{c: n_c>0} [ sumsq_c / n_c - ||s_c||^2 / n_c^2 ],
with U the number of non-empty classes. Empty classes contribute 0 in both
forms. This needs only ONE pass over the embeddings.

Device strategy (data-parallel over rows, all 100 classes per core):
  - one-hot H for each 128-row group via DVE is_equal(iota, label_col)
  - TensorE accumulates H^T @ E -> sums[C,D] and H^T @ [E^2 | 1] ->
    (per-dim sumsq | counts)[C,D+1] in PSUM across all groups
  - AllReduce the [C, 2D+1] partials over the 8 cores
  - tiny on-device reduction to the scalar loss
"""

import numpy as np

import concourse.bacc as bacc
import concourse.mybir as mybir
import concourse.tile as tile
from concourse.alu_op_type import AluOpType
from concourse.bass_utils import run_bass_kernel_spmd

N = 262144
D = 256
C = 100
N_CORES = 8
SHARD = N // N_CORES      # 32768 rows per core
P = 128                   # rows per matmul group (= SBUF partitions)
G = 8                     # groups per super-tile (1 MB DMAs)
ROWS_ST = P * G           # 1024 rows per super-tile
N_ST = SHARD // ROWS_ST   # 32 super-tiles per core
LCOLS = SHARD // P        # 256 label columns per core

_cache = {}


def _build():
    f32 = mybir.dt.float32
    bf16 = mybir.dt.bfloat16
    nc = bacc.Bacc("TRN2", target_bir_lowering=False, debug=False,
                   num_devices=N_CORES)
    emb = nc.dram_tensor("emb", [SHARD, D], f32, kind="ExternalInput")
    labt = nc.dram_tensor("labt", [P, LCOLS], f32, kind="ExternalInput")
    iota = nc.dram_tensor("iota", [P, P], bf16, kind="ExternalInput")
    loss = nc.dram_tensor("loss", [1, 1], f32, kind="ExternalOutput")

    # emb row (t*1024 + p*8 + g) -> super-tile t, partition p, group g.
    # Per partition the DMA source is 8 KB contiguous.
    emb_v = emb[:].rearrange("(t p g) d -> t p (g d)", p=P, g=G)

    with tile.TileContext(nc) as tc:
        with tc.tile_pool(name="const", bufs=1) as const, \
             tc.tile_pool(name="esup", bufs=4) as esup_pool, \
             tc.tile_pool(name="ebf", bufs=4) as ebf_pool, \
             tc.tile_pool(name="esq", bufs=4) as esq_pool, \
             tc.tile_pool(name="hpool", bufs=24) as hpool, \
             tc.tile_pool(name="psum", bufs=1, space="PSUM") as psum_pool, \
             tc.tile_pool(name="dram", bufs=1, space="DRAM") as dram_pool:

            iota_sb = const.tile([P, P], bf16)
            nc.sync.dma_start(iota_sb[:], iota[:])
            labt_sb = const.tile([P, LCOLS], f32)
            nc.sync.dma_start(labt_sb[:], labt[:])

            # lhsT H is [P, P] (one-hot padded to 128 classes, cols C..127
            # never match) so bf16 FWL kicks in; PSUM rows C..127 stay zero.
            psum_sums = psum_pool.tile([P, D], f32)       # H^T @ E
            psum_sq = psum_pool.tile([P, D + 1], f32)     # H^T @ [E^2 | 1]

            for t in range(N_ST):
                e_sup = esup_pool.tile([P, G * D], f32)
                nc.sync.dma_start(e_sup[:], emb_v[t])
                e_bf = ebf_pool.tile([P, G * D], bf16)
                nc.vector.tensor_copy(e_bf[:], e_sup[:])
                esq = esq_pool.tile([P, G, D + 1], bf16)
                nc.scalar.activation(
                    esq[:, :, 0:D],
                    e_sup[:].rearrange("p (g d) -> p g d", g=G),
                    mybir.ActivationFunctionType.Square,
                )
                nc.any.memset(esq[:, :, D], 1.0)
                for g in range(G):
                    h = hpool.tile([P, P], bf16)
                    nc.vector.tensor_scalar(
                        h[:], iota_sb[:],
                        labt_sb[:, t * G + g: t * G + g + 1],
                        None, AluOpType.is_equal,
                    )
                    first = t == 0 and g == 0
                    last = t == N_ST - 1 and g == G - 1
                    nc.tensor.matmul(psum_sums[:], h[:],
                                     e_bf[:, g * D:(g + 1) * D],
                                     start=first, stop=last)
                    nc.tensor.matmul(psum_sq[:], h[:], esq[:, g, :],
                                     start=first, stop=last)

            # Pack partial stats [C, 2D+1] = [sums | per-dim sumsq | counts]
            stats = const.tile([C, 2 * D + 1], f32)
            nc.vector.tensor_copy(stats[:, 0:D], psum_sums[:C, :])
            nc.vector.tensor_copy(stats[:, D:2 * D + 1], psum_sq[:C, :])

            partial = dram_pool.tile([C, 2 * D + 1], f32)
            allred = dram_pool.tile([C, 2 * D + 1], f32)
            nc.sync.dma_start(partial[:], stats[:])
            nc.gpsimd.collective_compute(
                "AllReduce", AluOpType.add,
                replica_groups=[list(range(N_CORES))],
                ins=[partial.opt()], outs=[allred.opt()],
            )
            red_sb = const.tile([C, 2 * D + 1], f32)
            nc.sync.dma_start(red_sb[:], allred[:])

            sums = red_sb[:, 0:D]
            sumsq_cols = red_sb[:, D:2 * D]
            counts = red_sb[:, 2 * D:2 * D + 1]

            sumsq = const.tile([C, 1], f32)
            nc.vector.tensor_reduce(sumsq[:], sumsq_cols,
                                    axis=mybir.AxisListType.X,
                                    op=AluOpType.add)
            sq_scr = const.tile([C, D], f32)
            s2 = const.tile([C, 1], f32)   # ||s_c||^2
            nc.scalar.activation(sq_scr[:], sums,
                                 mybir.ActivationFunctionType.Square,
                                 accum_out=s2[:])
            safe = const.tile([C, 1], f32)
            nc.vector.tensor_scalar_max(safe[:], counts, 1.0)
            inv = const.tile([C, 1], f32)
            nc.vector.reciprocal(inv[:], safe[:])

            # per-class loss = (sumsq - s2*inv) * inv ; 0 for empty classes
            pf = const.tile([C, 2], f32)
            t1 = const.tile([C, 1], f32)
            nc.vector.tensor_tensor(t1[:], s2[:], inv[:], AluOpType.mult)
            t2 = const.tile([C, 1], f32)
            nc.vector.tensor_tensor(t2[:], sumsq[:], t1[:], AluOpType.subtract)
            nc.vector.tensor_tensor(pf[:, 0:1], t2[:], inv[:], AluOpType.mult)
            nc.vector.tensor_scalar(pf[:, 1:2], counts, 0.0, None,
                                    AluOpType.is_gt)

            # partition-sum via ones^T @ [per | flag] -> [1, 2]
            ones_col = const.tile([P, 1], f32)
            nc.any.memset(ones_col[:], 1.0)
            fin_ps = psum_pool.tile([1, 2], f32)
            nc.tensor.matmul(fin_ps[:], ones_col[:C, :], pf[:],
                             start=True, stop=True)
            fin = const.tile([1, 2], f32)
            nc.vector.tensor_copy(fin[:], fin_ps[:])
            r = const.tile([1, 1], f32)
            nc.vector.reciprocal(r[:], fin[:, 1:2])
            out_sb = const.tile([1, 1], f32)
            nc.vector.tensor_tensor(out_sb[:], fin[:, 0:1], r[:],
                                    AluOpType.mult)
            nc.sync.dma_start(loss[:], out_sb[:])

    nc.compile()
    return nc


def _in_maps(embeddings: np.ndarray, labels: np.ndarray):
    import ml_dtypes
    bf16 = ml_dtypes.bfloat16
    emb = np.ascontiguousarray(np.asarray(embeddings), dtype=np.float32)
    lab = np.asarray(labels).astype(np.float32)
    iota = np.ascontiguousarray(
        np.broadcast_to(np.arange(P, dtype=np.float32), (P, P))).astype(bf16)
    maps = []
    for i in range(N_CORES):
        sl = slice(i * SHARD, (i + 1) * SHARD)
        labt = np.ascontiguousarray(
            lab[sl].reshape(N_ST, P, G).transpose(1, 0, 2).reshape(P, LCOLS))
        maps.append({"emb": emb[sl], "labt": labt, "iota": iota})
    return maps


def _run(embeddings: np.ndarray, labels: np.ndarray, trace: bool = False):
    if "nc" not in _cache:
        _cache["nc"] = _build()
    nc = _cache["nc"]
    res = run_bass_kernel_spmd(nc, _in_maps(embeddings, labels),
                               list(range(N_CORES)), trace=trace)
    out = np.float32(res.results[0]["loss"][0, 0])
    return out.reshape(()), res


def kernel(embeddings: np.ndarray, labels: np.ndarray) -> np.ndarray:
    out, _ = _run(embeddings, labels)
    return out
